# revision 17
# baseline (speedup 1.0000x reference)
"""Causal depthwise temporal conv (K=4) on 8 TRN2 NeuronCores.

Reference semantics (for x: [B, T, D], w: [K, D], b: [D]):
    out[bt, t, d] = sum_{j=0}^{K-1} x_pad[bt, t + j, d] * w[j, d] + b[d]
where x_pad is x left-padded with K-1 zeros along time.

Strategy (v8, fp16 + PE accumulation + balanced engines + tuned DMA):
  - Tensor-parallel over channels: core m owns channels [m*512, (m+1)*512).
  - All HBM traffic in fp16 (harness gate is rel_err < 2e-2; fp16 keeps
    max-rel error ~1e-3): halves DMA bytes vs f32. Single-core aggregate
    DMA is hard-capped at 358GB/s, so the wall floor is ~8us preamble +
    33.6MB/358GB/s + ~4us postamble ~= 106us.
  - Taps {0,1,3} run on the tensor engine as diagonal-matrix matmuls
    accumulating for free in PSUM (adds on DVE are the scarce resource:
    tensor_tensor is 2x-mode at best, scalar_tensor_tensor is 1x-only).
    ACT evacuates PSUM -> SBUF fp16 fused with the bias add. DVE does
    tap 2 (tensor_scalar, 4x) + one tensor_tensor combine (2x). On
    alternating halves one 512-col block of tap 0 moves to DVE (fused
    scalar_tensor_tensor) to shave the matmul stream without humps.
  - PSUM ping-pong: 2048-col halves, 4 banks each, bufs=2 = all 8 banks.
  - DMA: per-half 0.5MB loads (4096B rows, runt-free; fine-grained PE
    deps) on the sync HWDGE ring; per-chain 1MB stores on the gpsimd
    SWDGE queue (bigger transfers amortize the FIFO completion latency:
    269 vs 219GB/s measured); weights on the scalar ring so the ACT
    engine only runs ACTIVATEs. The last chains' stores alternate onto
    the sync ring (idle once loads finish) to drain the tail two-wide.
    First chain loads in 512-col pieces; last half drains in pieces.
"""

import numpy as np

import concourse.bacc as bacc
import concourse.mybir as mybir
from concourse.tile import TileContext
from concourse import bass_utils

B = 4            # batch
T = 4096         # sequence length
D = 4096         # channels (width)
K = 4            # temporal taps
N_CORES = 8
D_SH = D // N_CORES          # 512 channels per core
P = 128                      # SBUF partitions
N_BLK = D_SH // P            # 4 channel blocks per core
TPP = 4104                   # padded time length in DRAM (4096 + 8)
W_STRIDE = K + 1             # per-blk slot in the f32 wb tile

PE_TAPS = (0, 1, 3)          # taps done as diag-matmuls into PSUM
DVE_TAP = 2                  # tap done on DVE (even shift: 4B alignment)
HALF = 2048                  # psum half-chain width (4 banks)
NB = 512                     # matmul moving-block width (1 psum bank)
TAIL = 8                     # extra columns fetched for cross-half tap reads


def _build(b=B, t=T, n_blk=N_BLK):
    nc = bacc.Bacc("TRN2")
    f16 = mybir.dt.float16
    f32 = mybir.dt.float32
    npe = len(PE_TAPS)
    x = nc.dram_tensor("x", [n_blk, b, P, TPP], f16, kind="ExternalInput")
    wd = nc.dram_tensor("wd", [P, n_blk * npe * P], f16, kind="ExternalInput")
    wb = nc.dram_tensor("wb", [P, n_blk * W_STRIDE], f32, kind="ExternalInput")
    out = nc.dram_tensor("out", [n_blk, b, P, t], f16, kind="ExternalOutput")
    mult, add = mybir.AluOpType.mult, mybir.AluOpType.add
    ident = mybir.ActivationFunctionType.Identity
    nhalf = t // HALF
    hp = HALF + TAIL
    nchain = n_blk * b

    with TileContext(nc) as tc:
        with tc.tile_pool(name="px", bufs=6) as px, \
             tc.tile_pool(name="ps", bufs=4) as ps, \
             tc.tile_pool(name="po", bufs=3) as po, \
             tc.tile_pool(name="pw", bufs=1) as pw, \
             tc.tile_pool(name="pp", bufs=2, space="PSUM") as pp:
            # blk-0's diags ride first on the sync ring (tiny, 0.5us) so
            # the first matmul isn't weight-gated; the rest on scalar.
            wdt = pw.tile([P, n_blk * npe * P], f16, tag="wd")
            nc.sync.dma_start(wdt[:, 0:npe * P], wd[:, 0:npe * P])
            nc.scalar.dma_start(wdt[:, npe * P:], wd[:, npe * P:])
            wt = pw.tile([P, n_blk * W_STRIDE], f32, tag="wb")
            nc.scalar.dma_start(wt[:, :], wb[:, :])

            for blk in range(n_blk):
                def w(j, blk=blk):
                    return wt[:, blk * W_STRIDE + j:blk * W_STRIDE + j + 1]

                def wdiag(ti, blk=blk):
                    o = (blk * npe + ti) * P
                    return wdt[:, o:o + P]

                for bb in range(b):
                    chain = blk * b + bb
                    first = chain == 0
                    last = chain == nchain - 1
                    o = po.tile([P, t], f16, tag="o")
                    for h in range(nhalf):
                        base = h * HALF
                        # DVE takes tap0's last 512-col block on alternating
                        # halves (smooth PE/DVE rebalance, no humps).
                        reb = (chain * nhalf + h) % 2 == 0 and \
                            not (last and h == nhalf - 1)
                        # Runt-free per-half load + 16B-row tail; the very
                        # first load lands in 512-col pieces.
                        X = px.tile([P, hp], f16, tag="x")
                        # Before the first stores exist (~25us) the gpsimd
                        # queue is idle: odd early halves load there so the
                        # early phase isn't capped by one ring.
                        ldq = nc.gpsimd if chain < 2 and h == 1 else nc.sync
                        if first and h == 0:
                            for c in range(HALF // NB):
                                nc.sync.dma_start(
                                    X[:, c * NB:(c + 1) * NB],
                                    x[blk, bb, :, c * NB:(c + 1) * NB])
                        else:
                            ldq.dma_start(X[:, 0:HALF],
                                          x[blk, bb, :, base:base + HALF])
                        ldq.dma_start(X[:, HALF:hp],
                                      x[blk, bb, :, base + HALF:base + hp])
                        pt = pp.tile([P, HALF], f32, tag="ps")
                        # PE: psum[c] += sum_j diag(w_j) @ x_j. Tap-major;
                        # first half block-major to chase its piecewise load.
                        mmlist = []
                        for ti in range(npe):
                            for c in range(HALF // NB):
                                if reb and ti == 0 and c == HALF // NB - 1:
                                    continue    # this block's tap0 -> DVE
                                mmlist.append((c, ti))
                        if first and h == 0:
                            mmlist.sort(key=lambda cti: cti[0])
                        # start/stop per psum region (first/last visit).
                        seen = {}
                        lastv = {}
                        for idx, (c, ti) in enumerate(mmlist):
                            lastv[c] = idx
                        for idx, (c, ti) in enumerate(mmlist):
                            lo = c * NB + PE_TAPS[ti]
                            nc.tensor.matmul(
                                pt[:, c * NB:(c + 1) * NB],
                                wdiag(ti),
                                X[:, lo:lo + NB],
                                start=c not in seen,
                                stop=lastv[c] == idx,
                                skip_group_check=True,
                            )
                            seen[c] = True
                        # Evac + combine; last half in 512-col pieces.
                        pieces = 4 if (last and h == nhalf - 1) else 1
                        pw_ = HALF // pieces
                        for q in range(pieces):
                            qs = q * pw_
                            s = ps.tile([P, pw_], f16, tag="s")
                            nc.scalar.activation(s[:], pt[:, qs:qs + pw_],
                                                 ident, bias=w(K), scale=1.0)
                            y = ps.tile([P, pw_], f16, tag="y")
                            nc.vector.tensor_scalar_mul(
                                y[:], X[:, qs + DVE_TAP:qs + DVE_TAP + pw_],
                                w(DVE_TAP))
                            if reb:
                                # fold the moved tap0 block: z = w0*x0 + s
                                # for the last 512 cols, then combine the
                                # two column ranges separately.
                                rb = HALF - NB
                                z = ps.tile([P, NB], f16, tag="z")
                                nc.vector.scalar_tensor_tensor(
                                    z[:], X[:, rb:HALF], w(0),
                                    s[:, rb:HALF], mult, add)
                                nc.vector.tensor_tensor(
                                    o[:, base:base + rb],
                                    y[:, 0:rb], s[:, 0:rb], add)
                                nc.vector.tensor_tensor(
                                    o[:, base + rb:base + HALF],
                                    y[:, rb:HALF], z[:], add)
                            else:
                                nc.vector.tensor_tensor(
                                    o[:, base + qs:base + qs + pw_],
                                    y[:], s[:], add)
                            if last and h == nhalf - 1:
                                stq = nc.gpsimd if q % 2 == 0 else nc.sync
                                stq.dma_start(
                                    out[blk, bb, :,
                                        base + qs:base + qs + pw_],
                                    o[:, base + qs:base + qs + pw_])
                        if last and h < nhalf - 1:
                            nc.gpsimd.dma_start(
                                out[blk, bb, :, base:base + HALF],
                                o[:, base:base + HALF])
                    # Per-chain 1MB store (runt-free 8192B rows) on the
                    # gpsimd SWDGE queue; near the end alternate onto the
                    # sync ring, which is idle once loads are done.
                    if not last:
                        stq = nc.sync if chain >= nchain - 3 and \
                            chain % 2 == 0 else nc.gpsimd
                        stq.dma_start(out[blk, bb, :, :], o[:, :])
    nc.compile()
    return nc


def _prepare(x, w, b):
    x = np.asarray(x, dtype=np.float32)
    w = np.asarray(w, dtype=np.float32)
    b = np.asarray(b, dtype=np.float32)
    npe = len(PE_TAPS)
    # channel-major, left zero-padded time, fp16: [D, B, TPP]
    xp = np.zeros((D, B, TPP), dtype=np.float16)
    xp[:, :, K - 1:K - 1 + T] = x.transpose(2, 0, 1)
    wbt = np.concatenate([w.T, b[:, None]], axis=1).astype(np.float32)  # [D, 5]
    in_maps = []
    for m in range(N_CORES):
        sl = slice(m * D_SH, (m + 1) * D_SH)
        wbm = wbt[sl].reshape(N_BLK, P, W_STRIDE).transpose(1, 0, 2)
        # diag stationary matrices for the PE taps: [P, n_blk*npe*P]
        wdm = np.zeros((P, N_BLK, npe, P), dtype=np.float16)
        rng = np.arange(P)
        for blk in range(N_BLK):
            for ti, tap in enumerate(PE_TAPS):
                wdm[rng, blk, ti, rng] = w[tap, m * D_SH + blk * P + rng]
        in_maps.append({
            "x": np.ascontiguousarray(
                xp[sl].reshape(N_BLK, P, B, TPP).transpose(0, 2, 1, 3)),
            "wd": np.ascontiguousarray(wdm).reshape(P, N_BLK * npe * P),
            "wb": np.ascontiguousarray(wbm).reshape(P, N_BLK * W_STRIDE),
        })
    return in_maps


def _collect(results):
    out = np.empty((B, T, D), dtype=np.float32)
    for m in range(N_CORES):
        o = np.asarray(results[m]["out"]).astype(np.float32)
        o = o.reshape(N_BLK, B, P, T).transpose(1, 3, 0, 2).reshape(B, T, D_SH)
        out[:, :, m * D_SH:(m + 1) * D_SH] = o
    return out


def _run(in_maps, trace=False, **kwargs):
    nc = _build()
    return bass_utils.run_bass_kernel_spmd(
        nc, in_maps, core_ids=list(range(N_CORES)), trace=trace, **kwargs)


def kernel(x, w, b):
    in_maps = _prepare(x, w, b)
    try:
        res = _run(in_maps)
    except Exception:
        # Transient NRT device errors have been observed on a cold first
        # execute; one retry (fresh compile dir) clears them.
        res = _run(in_maps)
    return _collect(res.results)


# revision 18
# speedup vs baseline: 1.0436x; 1.0436x over previous
"""Causal depthwise temporal conv (K=4) on 8 TRN2 NeuronCores.

Reference semantics (for x: [B, T, D], w: [K, D], b: [D]):
    out[bt, t, d] = sum_{j=0}^{K-1} x_pad[bt, t + j, d] * w[j, d] + b[d]
where x_pad is x left-padded with K-1 zeros along time.

Strategy (v8, fp16 + PE accumulation + balanced engines + tuned DMA):
  - Tensor-parallel over channels: core m owns channels [m*512, (m+1)*512).
  - All HBM traffic in fp16 (harness gate is rel_err < 2e-2; fp16 keeps
    max-rel error ~1e-3): halves DMA bytes vs f32. Single-core aggregate
    DMA is hard-capped at 358GB/s, so the wall floor is ~8us preamble +
    33.6MB/358GB/s + ~4us postamble ~= 106us.
  - Taps {0,1,3} run on the tensor engine as diagonal-matrix matmuls
    accumulating for free in PSUM (adds on DVE are the scarce resource:
    tensor_tensor is 2x-mode at best, scalar_tensor_tensor is 1x-only).
    ACT evacuates PSUM -> SBUF fp16 fused with the bias add. DVE does
    tap 2 (tensor_scalar, 4x) + one tensor_tensor combine (2x). On
    alternating halves one 512-col block of tap 0 moves to DVE (fused
    scalar_tensor_tensor) to shave the matmul stream without humps.
  - PSUM ping-pong: 2048-col halves, 4 banks each, bufs=2 = all 8 banks.
  - DMA: per-half 0.5MB loads (4096B rows, runt-free; fine-grained PE
    deps) on the sync HWDGE ring; per-chain 1MB stores on the gpsimd
    SWDGE queue (bigger transfers amortize the FIFO completion latency:
    269 vs 219GB/s measured); weights on the scalar ring so the ACT
    engine only runs ACTIVATEs. The last chains' stores alternate onto
    the sync ring (idle once loads finish) to drain the tail two-wide.
    First chain loads in 512-col pieces; last half drains in pieces.
"""

import numpy as np

import concourse.bacc as bacc
import concourse.mybir as mybir
from concourse.tile import TileContext
from concourse import bass_utils

B = 4            # batch
T = 4096         # sequence length
D = 4096         # channels (width)
K = 4            # temporal taps
N_CORES = 8
D_SH = D // N_CORES          # 512 channels per core
P = 128                      # SBUF partitions
N_BLK = D_SH // P            # 4 channel blocks per core
TPP = 4104                   # padded time length in DRAM (4096 + 8)
W_STRIDE = K + 1             # per-blk slot in the f32 wb tile

PE_TAPS = (0, 1, 3)          # taps done as diag-matmuls into PSUM
DVE_TAP = 2                  # tap done on DVE (even shift: 4B alignment)
HALF = 2048                  # psum half-chain width (4 banks)
NB = 512                     # matmul moving-block width (1 psum bank)
TAIL = 8                     # extra columns fetched for cross-half tap reads


def _build(b=B, t=T, n_blk=N_BLK):
    nc = bacc.Bacc("TRN2")
    f16 = mybir.dt.float16
    f32 = mybir.dt.float32
    npe = len(PE_TAPS)
    x = nc.dram_tensor("x", [n_blk, b, P, TPP], f16, kind="ExternalInput")
    wd = nc.dram_tensor("wd", [P, n_blk * npe * P], f16, kind="ExternalInput")
    wb = nc.dram_tensor("wb", [P, n_blk * W_STRIDE], f32, kind="ExternalInput")
    out = nc.dram_tensor("out", [n_blk, b, P, t], f16, kind="ExternalOutput")
    mult, add = mybir.AluOpType.mult, mybir.AluOpType.add
    ident = mybir.ActivationFunctionType.Identity
    nhalf = t // HALF
    hp = HALF + TAIL
    nchain = n_blk * b

    with TileContext(nc) as tc:
        with tc.tile_pool(name="px", bufs=6) as px, \
             tc.tile_pool(name="ps", bufs=4) as ps, \
             tc.tile_pool(name="po", bufs=3) as po, \
             tc.tile_pool(name="pw", bufs=1) as pw, \
             tc.tile_pool(name="pp", bufs=2, space="PSUM") as pp:
            # blk-0's diags ride first on the sync ring (tiny, 0.5us) so
            # the first matmul isn't weight-gated; the rest on scalar.
            wdt = pw.tile([P, n_blk * npe * P], f16, tag="wd")
            nc.sync.dma_start(wdt[:, 0:npe * P], wd[:, 0:npe * P])
            nc.scalar.dma_start(wdt[:, npe * P:], wd[:, npe * P:])
            wt = pw.tile([P, n_blk * W_STRIDE], f32, tag="wb")
            nc.scalar.dma_start(wt[:, :], wb[:, :])

            for blk in range(n_blk):
                def w(j, blk=blk):
                    return wt[:, blk * W_STRIDE + j:blk * W_STRIDE + j + 1]

                def wdiag(ti, blk=blk):
                    o = (blk * npe + ti) * P
                    return wdt[:, o:o + P]

                for bb in range(b):
                    chain = blk * b + bb
                    first = chain == 0
                    last = chain == nchain - 1
                    o = po.tile([P, t], f16, tag="o")
                    for h in range(nhalf):
                        base = h * HALF
                        # DVE takes tap0's last 512-col block on alternating
                        # halves (smooth PE/DVE rebalance, no humps).
                        reb = (chain * nhalf + h) % 2 == 0 and \
                            not (last and h == nhalf - 1)
                        # Runt-free per-half load + 16B-row tail; the very
                        # first load lands in 512-col pieces.
                        X = px.tile([P, hp], f16, tag="x")
                        if first and h == 0:
                            for c in range(HALF // NB):
                                nc.sync.dma_start(
                                    X[:, c * NB:(c + 1) * NB],
                                    x[blk, bb, :, c * NB:(c + 1) * NB])
                        else:
                            nc.sync.dma_start(X[:, 0:HALF],
                                              x[blk, bb, :, base:base + HALF])
                        nc.sync.dma_start(X[:, HALF:hp],
                                          x[blk, bb, :, base + HALF:base + hp])
                        pt = pp.tile([P, HALF], f32, tag="ps")
                        # PE: psum[c] += sum_j diag(w_j) @ x_j. Tap-major;
                        # first half block-major to chase its piecewise load.
                        mmlist = []
                        for ti in range(npe):
                            for c in range(HALF // NB):
                                if reb and ti == 0 and c == HALF // NB - 1:
                                    continue    # this block's tap0 -> DVE
                                mmlist.append((c, ti))
                        if first and h == 0:
                            mmlist.sort(key=lambda cti: cti[0])
                        # start/stop per psum region (first/last visit).
                        seen = {}
                        lastv = {}
                        for idx, (c, ti) in enumerate(mmlist):
                            lastv[c] = idx
                        for idx, (c, ti) in enumerate(mmlist):
                            lo = c * NB + PE_TAPS[ti]
                            nc.tensor.matmul(
                                pt[:, c * NB:(c + 1) * NB],
                                wdiag(ti),
                                X[:, lo:lo + NB],
                                start=c not in seen,
                                stop=lastv[c] == idx,
                                skip_group_check=True,
                            )
                            seen[c] = True
                        # Evac + combine; last half in 512-col pieces.
                        pieces = 4 if (last and h == nhalf - 1) else 1
                        pw_ = HALF // pieces
                        for q in range(pieces):
                            qs = q * pw_
                            s = ps.tile([P, pw_], f16, tag="s")
                            nc.scalar.activation(s[:], pt[:, qs:qs + pw_],
                                                 ident, bias=w(K), scale=1.0)
                            y = ps.tile([P, pw_], f16, tag="y")
                            nc.vector.tensor_scalar_mul(
                                y[:], X[:, qs + DVE_TAP:qs + DVE_TAP + pw_],
                                w(DVE_TAP))
                            if reb:
                                # fold the moved tap0 block: z = w0*x0 + s
                                # for the last 512 cols, then combine the
                                # two column ranges separately.
                                rb = HALF - NB
                                z = ps.tile([P, NB], f16, tag="z")
                                nc.vector.scalar_tensor_tensor(
                                    z[:], X[:, rb:HALF], w(0),
                                    s[:, rb:HALF], mult, add)
                                nc.vector.tensor_tensor(
                                    o[:, base:base + rb],
                                    y[:, 0:rb], s[:, 0:rb], add)
                                nc.vector.tensor_tensor(
                                    o[:, base + rb:base + HALF],
                                    y[:, rb:HALF], z[:], add)
                            else:
                                nc.vector.tensor_tensor(
                                    o[:, base + qs:base + qs + pw_],
                                    y[:], s[:], add)
                            if last and h == nhalf - 1:
                                stq = nc.gpsimd if q % 2 == 0 else nc.sync
                                stq.dma_start(
                                    out[blk, bb, :,
                                        base + qs:base + qs + pw_],
                                    o[:, base + qs:base + qs + pw_])
                        if last and h < nhalf - 1:
                            nc.gpsimd.dma_start(
                                out[blk, bb, :, base:base + HALF],
                                o[:, base:base + HALF])
                    # Per-chain 1MB store (runt-free 8192B rows) on the
                    # gpsimd SWDGE queue; near the end alternate onto the
                    # sync ring, which is idle once loads are done.
                    if not last:
                        stq = nc.sync if chain >= nchain - 3 and \
                            chain % 2 == 0 else nc.gpsimd
                        stq.dma_start(out[blk, bb, :, :], o[:, :])
    nc.compile()
    return nc


def _prepare(x, w, b):
    x = np.asarray(x, dtype=np.float32)
    w = np.asarray(w, dtype=np.float32)
    b = np.asarray(b, dtype=np.float32)
    npe = len(PE_TAPS)
    # channel-major, left zero-padded time, fp16: [D, B, TPP]
    xp = np.zeros((D, B, TPP), dtype=np.float16)
    xp[:, :, K - 1:K - 1 + T] = x.transpose(2, 0, 1)
    wbt = np.concatenate([w.T, b[:, None]], axis=1).astype(np.float32)  # [D, 5]
    in_maps = []
    for m in range(N_CORES):
        sl = slice(m * D_SH, (m + 1) * D_SH)
        wbm = wbt[sl].reshape(N_BLK, P, W_STRIDE).transpose(1, 0, 2)
        # diag stationary matrices for the PE taps: [P, n_blk*npe*P]
        wdm = np.zeros((P, N_BLK, npe, P), dtype=np.float16)
        rng = np.arange(P)
        for blk in range(N_BLK):
            for ti, tap in enumerate(PE_TAPS):
                wdm[rng, blk, ti, rng] = w[tap, m * D_SH + blk * P + rng]
        in_maps.append({
            "x": np.ascontiguousarray(
                xp[sl].reshape(N_BLK, P, B, TPP).transpose(0, 2, 1, 3)),
            "wd": np.ascontiguousarray(wdm).reshape(P, N_BLK * npe * P),
            "wb": np.ascontiguousarray(wbm).reshape(P, N_BLK * W_STRIDE),
        })
    return in_maps


def _collect(results):
    out = np.empty((B, T, D), dtype=np.float32)
    for m in range(N_CORES):
        o = np.asarray(results[m]["out"]).astype(np.float32)
        o = o.reshape(N_BLK, B, P, T).transpose(1, 3, 0, 2).reshape(B, T, D_SH)
        out[:, :, m * D_SH:(m + 1) * D_SH] = o
    return out


def _run(in_maps, trace=False, **kwargs):
    nc = _build()
    return bass_utils.run_bass_kernel_spmd(
        nc, in_maps, core_ids=list(range(N_CORES)), trace=trace, **kwargs)


def kernel(x, w, b):
    in_maps = _prepare(x, w, b)
    try:
        res = _run(in_maps)
    except Exception:
        # Transient NRT device errors have been observed on a cold first
        # execute; one retry (fresh compile dir) clears them.
        res = _run(in_maps)
    return _collect(res.results)


# revision 19
# speedup vs baseline: 1.0548x; 1.0107x over previous
"""Causal depthwise temporal conv (K=4) on 8 TRN2 NeuronCores.

Reference semantics (for x: [B, T, D], w: [K, D], b: [D]):
    out[bt, t, d] = sum_{j=0}^{K-1} x_pad[bt, t + j, d] * w[j, d] + b[d]
where x_pad is x left-padded with K-1 zeros along time.

Strategy (fp16 + PE accumulation + 3-queue DMA):
  - Tensor-parallel over channels: core m owns channels [m*512, (m+1)*512)
    (the conv is depthwise, channels fully independent, no collectives).
  - All HBM traffic in fp16 (the harness gate is rel_err < 2e-2; fp16
    keeps max-rel error ~8e-4): halves DMA bytes vs f32. Per-core traffic
    is 16.8MB in + 16.8MB out; single-core aggregate DMA is hard-capped
    at ~358GB/s, so the wall floor is ~8us preamble + 94us DMA + ~4us
    postamble. This kernel measures ~109us.
  - Compute split (per 128-channel block x batch chain, channels on SBUF
    partitions so per-channel weights are per-partition scalars):
      * Taps {0,1,3} run on the tensor engine as diagonal-matrix matmuls
        accumulating for free in PSUM (diag(w_j).T @ x_shift_j). The adds
        are the scarce resource on DVE - tensor_tensor is 2x-mode at
        best and scalar_tensor_tensor is 1x-only (measured) - so PSUM
        accumulation wins. Odd shifts break DVE's 2x-mode 4B-alignment
        rule anyway; the PE doesn't care.
      * ACT evacuates PSUM -> SBUF fp16 fused with the bias add (1x but
        alignment/dtype-insensitive, and ScalarE is otherwise idle).
      * DVE does tap 2 (tensor_scalar, 4x mode) and one tensor_tensor
        combine (2x mode). Engine busy: PE ~87us, ACT ~63us, DVE ~63us,
        all under the ~95us DMA wall.
  - PSUM ping-pong: each chain is split into two 2048-col halves; each
    half uses 4 PSUM banks, bufs=2 covers all 8 banks, so the PE fills
    one half while ACT drains the other. Matmul stream measures ~219ns
    per 512-col matmul (dense, zero gaps; LDWEIGHTS fully hidden by the
    PE's reorder window).
  - DMA: loads on the sync HWDGE ring as exact-4096B rows (a 2051-elem
    row is 4102B -> per-row runt packet, measured 15% slower) plus a tiny
    16B-row tail transfer per half; stores per-half on the gpsimd SWDGE
    queue; weights on the sync ring ahead of the first load. Three queues
    overlap to reach the 358GB/s cap; the ACT engine only runs ACTIVATEs
    (store dispatches on its queue would stall PSUM evacuation).
  - Edges: the first half loads in 512-col pieces and runs its matmuls
    block-major (PE starts ~2us sooner); the last half evacuates and
    stores in 512-col pieces (short drain tail).
"""

import numpy as np

import concourse.bacc as bacc
import concourse.mybir as mybir
from concourse.tile import TileContext
from concourse import bass_utils

B = 4            # batch
T = 4096         # sequence length
D = 4096         # channels (width)
K = 4            # temporal taps
N_CORES = 8
D_SH = D // N_CORES          # 512 channels per core
P = 128                      # SBUF partitions
N_BLK = D_SH // P            # 4 channel blocks per core
TPP = 4104                   # padded time length in DRAM (4096 + 8)
W_STRIDE = K + 1             # per-blk slot in the f32 wb tile

PE_TAPS = (0, 1, 3)          # taps done as diag-matmuls into PSUM
DVE_TAPS = (2,)              # taps done on DVE (even shifts: 4B alignment)
HALF = 2048                  # psum half-chain width (4 banks)
NB = 512                     # matmul moving-block width (1 psum bank)
TAIL = 8                     # extra columns fetched for cross-half tap reads


def _build(b=B, t=T, n_blk=N_BLK):
    nc = bacc.Bacc("TRN2")
    f16 = mybir.dt.float16
    f32 = mybir.dt.float32
    npe = len(PE_TAPS)
    x = nc.dram_tensor("x", [n_blk, b, P, TPP], f16, kind="ExternalInput")
    wd = nc.dram_tensor("wd", [P, n_blk * npe * P], f16, kind="ExternalInput")
    wb = nc.dram_tensor("wb", [P, n_blk * W_STRIDE], f32, kind="ExternalInput")
    out = nc.dram_tensor("out", [n_blk, b, P, t], f16, kind="ExternalOutput")
    mult, add = mybir.AluOpType.mult, mybir.AluOpType.add
    ident = mybir.ActivationFunctionType.Identity
    nhalf = t // HALF
    hp = HALF + TAIL

    with TileContext(nc) as tc:
        with tc.tile_pool(name="px", bufs=6) as px, \
             tc.tile_pool(name="ps", bufs=4) as ps, \
             tc.tile_pool(name="po", bufs=4) as po, \
             tc.tile_pool(name="pw", bufs=1) as pw, \
             tc.tile_pool(name="pp", bufs=2, space="PSUM") as pp:
            wdt = pw.tile([P, n_blk * npe * P], f16, tag="wd")
            nc.sync.dma_start(wdt[:], wd[:, :])
            wt = pw.tile([P, n_blk * W_STRIDE], f32, tag="wb")
            nc.sync.dma_start(wt[:], wb[:, :])

            for blk in range(n_blk):
                def w(j, blk=blk):
                    return wt[:, blk * W_STRIDE + j:blk * W_STRIDE + j + 1]

                def wdiag(ti, blk=blk):
                    o = (blk * npe + ti) * P
                    return wdt[:, o:o + P]

                for bb in range(b):
                    for h in range(nhalf):
                        base = h * HALF
                        first = blk == 0 and bb == 0 and h == 0
                        last = blk == n_blk - 1 and bb == b - 1 \
                            and h == nhalf - 1
                        # Runt-free load: 4096B rows + 16B-row tail. The
                        # very first load lands in 512-col pieces so the PE
                        # starts ~2us sooner.
                        X = px.tile([P, hp], f16, tag="x")
                        if first:
                            for c in range(HALF // NB):
                                nc.sync.dma_start(
                                    X[:, c * NB:(c + 1) * NB],
                                    x[blk, bb, :, base + c * NB:
                                      base + (c + 1) * NB])
                        else:
                            nc.sync.dma_start(X[:, 0:HALF],
                                              x[blk, bb, :, base:base + HALF])
                        nc.sync.dma_start(X[:, HALF:hp],
                                          x[blk, bb, :, base + HALF:base + hp])
                        pt = pp.tile([P, HALF], f32, tag="ps")
                        # PE: psum[c] += sum_{j in PE_TAPS} diag(w_j) @ x_j
                        # Tap-major: tap-0 matmuls depend only on the main
                        # load, so the PE starts before the tail arrives.
                        # The first half runs block-major so matmuls chase
                        # the piecewise load.
                        order = [(c, ti) for c in range(HALF // NB)
                                 for ti in range(npe)] if first else \
                                [(c, ti) for ti in range(npe)
                                 for c in range(HALF // NB)]
                        for c, ti in order:
                            lo = c * NB + PE_TAPS[ti]
                            nc.tensor.matmul(
                                pt[:, c * NB:(c + 1) * NB],
                                wdiag(ti),
                                X[:, lo:lo + NB],
                                start=(ti == 0),
                                stop=(ti == npe - 1),
                                skip_group_check=True,
                            )
                        # Evac + combine + store; the last half drains in
                        # 512-col pieces so the tail is short.
                        pieces = 4 if last else 1
                        pw_ = HALF // pieces
                        for q in range(pieces):
                            qs = q * pw_
                            # ACT: evacuate PSUM -> fp16, fused bias add.
                            s = ps.tile([P, pw_], f16, tag="s")
                            nc.scalar.activation(s[:], pt[:, qs:qs + pw_],
                                                 ident, bias=w(K), scale=1.0)
                            # DVE: remaining even tap + combine (4x / 2x).
                            y = ps.tile([P, pw_], f16, tag="y")
                            j0 = DVE_TAPS[0]
                            nc.vector.tensor_scalar_mul(
                                y[:], X[:, qs + j0:qs + j0 + pw_], w(j0))
                            o = po.tile([P, pw_], f16, tag="o")
                            nc.vector.tensor_tensor(o[:], y[:], s[:], add)
                            # Stores ride the idle gpsimd SWDGE queue.
                            nc.gpsimd.dma_start(
                                out[blk, bb, :, base + qs:base + qs + pw_],
                                o[:])
    nc.compile()
    return nc


def _prepare(x, w, b):
    x = np.asarray(x, dtype=np.float32)
    w = np.asarray(w, dtype=np.float32)
    b = np.asarray(b, dtype=np.float32)
    npe = len(PE_TAPS)
    # channel-major, left zero-padded time, fp16: [D, B, TPP]
    xp = np.zeros((D, B, TPP), dtype=np.float16)
    xp[:, :, K - 1:K - 1 + T] = x.transpose(2, 0, 1)
    wbt = np.concatenate([w.T, b[:, None]], axis=1).astype(np.float32)  # [D, 5]
    in_maps = []
    for m in range(N_CORES):
        sl = slice(m * D_SH, (m + 1) * D_SH)
        wbm = wbt[sl].reshape(N_BLK, P, W_STRIDE).transpose(1, 0, 2)
        # diag stationary matrices for the PE taps: [P, n_blk*npe*P]
        wdm = np.zeros((P, N_BLK, npe, P), dtype=np.float16)
        rng = np.arange(P)
        for blk in range(N_BLK):
            for ti, tap in enumerate(PE_TAPS):
                wdm[rng, blk, ti, rng] = w[tap, m * D_SH + blk * P + rng]
        in_maps.append({
            "x": np.ascontiguousarray(
                xp[sl].reshape(N_BLK, P, B, TPP).transpose(0, 2, 1, 3)),
            "wd": np.ascontiguousarray(wdm).reshape(P, N_BLK * npe * P),
            "wb": np.ascontiguousarray(wbm).reshape(P, N_BLK * W_STRIDE),
        })
    return in_maps


def _collect(results):
    out = np.empty((B, T, D), dtype=np.float32)
    for m in range(N_CORES):
        o = np.asarray(results[m]["out"]).astype(np.float32)
        o = o.reshape(N_BLK, B, P, T).transpose(1, 3, 0, 2).reshape(B, T, D_SH)
        out[:, :, m * D_SH:(m + 1) * D_SH] = o
    return out


def _run(in_maps, trace=False, **kwargs):
    nc = _build()
    return bass_utils.run_bass_kernel_spmd(
        nc, in_maps, core_ids=list(range(N_CORES)), trace=trace, **kwargs)


def kernel(x, w, b):
    in_maps = _prepare(x, w, b)
    try:
        res = _run(in_maps)
    except Exception:
        # Transient NRT device errors have been observed on a cold first
        # execute; one retry (fresh compile dir) clears them.
        res = _run(in_maps)
    return _collect(res.results)
